# revision 1
# baseline (speedup 1.0000x reference)
"""IterNorm (iterative whitening normalization) Bass kernel for 8 TRN2 cores.

Reference (hardcoded shapes): X (64, 256, 56, 56) f32; g=4 groups of d=64
channels; m = 64*56*56 = 200704; Sigma = eps*I + (1/m) xc xc^T per group;
5 Newton-Schulz iters -> whitening wm; out = (wm @ xc) * weight + bias.

Sharding: data-parallel over batch B (8 b's per core). Per core:
  All x data is cast to bf16 on load (SWDGE cast DMA) and stays resident in
  SBUF (16 tiles of [128, 3136] = 98 KiB/partition), so HBM traffic is one
  f32 read + one f32 write of the shard. Channel-half h=0 (groups 0,1)
  streams first: PE-transpose bf16 chunks -> PSUM -> bf16 st tiles with a
  pre-primed ones column, so the covariance matmul (N=129) also accumulates
  per-channel sums. The h=0 stats are all-reduced + Newton-Schulz'd while
  h=1 still streams; the h=0 apply (one 128-wide block-diagonal bf16 matmul
  per 448-col chunk, one 1.6 MB store per tile) overlaps the h=1 collective
  + NS. Stats/NS math stays f32: PE for the small matmuls, ACT for
  PSUM->SBUF copies, GpSimd for SBUF elementwise — so the streaming DVE
  copy pipeline is never blocked by the latency-bound stats chain.
"""

import numpy as np

B, C, H, W = 64, 256, 56, 56
HW = H * W               # 3136
G, D = 4, 64             # groups, channels/group
NCORES = 8
BS = B // NCORES         # 8 batches per core
M = B * HW               # 200704 (full reduction length)
EPS = 1e-5
T_ITERS = 5

NCH = 128                # transpose chunk width (hw samples per chunk)
FULL_CHUNKS = HW // NCH  # 24
TAIL = HW - FULL_CHUNKS * NCH  # 64
NCHUNK = FULL_CHUNKS + 1       # 25
GRP = 8                  # chunks per psum/st group
APPLY_N = 448            # apply matmul free dim; 7 * 448 = 3136
ST_BUFS = 3
STAGE_BUFS = 3

_CACHE = {}


def _build_nc(single_core_sim=False, repeat=1):
    import concourse.bacc as bacc
    import concourse.tile as tile
    from concourse import mybir

    f32 = mybir.dt.float32
    bf16 = mybir.dt.bfloat16
    AX = mybir.AxisListType.X
    ADD = mybir.AluOpType.add
    SUB = mybir.AluOpType.subtract
    MULT = mybir.AluOpType.mult
    DIV = mybir.AluOpType.divide

    nc = bacc.Bacc(
        "TRN2",
        target_bir_lowering=False,
        debug=False,
        enable_asserts=False,
        num_devices=1 if single_core_sim else NCORES,
    )
    Xd = nc.dram_tensor("X", [BS, C, HW], f32, kind="ExternalInput").ap()
    Wd = nc.dram_tensor("weight", [C], f32, kind="ExternalInput").ap()
    Bd = nc.dram_tensor("bias", [C], f32, kind="ExternalInput").ap()
    Od = nc.dram_tensor("out", [BS, C, HW], f32, kind="ExternalOutput").ap()

    blksl = [slice(0, 64), slice(64, 128)]
    widths = [NCH] * FULL_CHUNKS + [TAIL]
    offs = [i * NCH for i in range(NCHUNK)]
    groups = [list(range(g0, min(g0 + GRP, NCHUNK)))
              for g0 in range(0, NCHUNK, GRP)]  # [8, 8, 8, 1]
    inv_m = 1.0 / float(M)

    with tile.TileContext(nc) as tc:
        with (
            tc.tile_pool(name="consts", bufs=1) as consts,
            tc.tile_pool(name="res", bufs=1) as res,
            tc.tile_pool(name="stp", bufs=ST_BUFS) as stp,
            tc.tile_pool(name="statsp", bufs=1) as statsp,
            tc.tile_pool(name="nss", bufs=1) as nss,
            tc.tile_pool(name="stg", bufs=STAGE_BUFS) as stg,
            tc.tile_pool(name="dram", bufs=1, space="DRAM") as dram,
            tc.tile_pool(name="trp", bufs=3, space="PSUM") as trp,
            tc.tile_pool(name="covp", bufs=1, space="PSUM") as covp,
            tc.tile_pool(name="nsp", bufs=2, space="PSUM") as nsp,
            tc.tile_pool(name="app", bufs=2, space="PSUM") as app,
        ):
            # ---- constants ----
            id_np = np.eye(128, dtype=np.float32)
            gm_np = np.zeros((128, 2), dtype=np.float32)
            gm_np[0:64, 0] = 1.0
            gm_np[64:128, 1] = 1.0
            identity_d = nc.inline_tensor(id_np, name="identity_c")
            id15_d = nc.inline_tensor(1.5 * id_np, name="id15_c")
            epsI_d = nc.inline_tensor(EPS * id_np, name="epsI_c")
            gmask_d = nc.inline_tensor(gm_np, name="gmask_c")
            ones_d = nc.inline_tensor(np.ones((1, 128), dtype=np.float32),
                                      name="ones_c")
            ones_col_d = nc.inline_tensor(np.ones((128, 1), dtype=np.float32),
                                          name="ones_col_c")
            gmaskT_d = nc.inline_tensor(np.ascontiguousarray(gm_np.T),
                                        name="gmaskT_c")
            identity = consts.tile([128, 128], f32)
            nc.sync.dma_start(out=identity, in_=identity_d.ap())
            id15 = consts.tile([128, 128], f32)
            nc.sync.dma_start(out=id15, in_=id15_d.ap())
            epsI = consts.tile([128, 128], f32)
            nc.sync.dma_start(out=epsI, in_=epsI_d.ap())
            gmask = consts.tile([128, 2], f32)
            nc.sync.dma_start(out=gmask, in_=gmask_d.ap())
            ones_row = consts.tile([1, 128], f32)
            nc.sync.dma_start(out=ones_row, in_=ones_d.ap())
            ones_col = consts.tile([128, 1], f32)
            nc.sync.dma_start(out=ones_col, in_=ones_col_d.ap())
            gmaskT = consts.tile([2, 128], f32)
            nc.sync.dma_start(out=gmaskT, in_=gmaskT_d.ap())
            wrow = consts.tile([1, C], f32)
            nc.sync.dma_start(out=wrow, in_=Wd[None, :])
            bcol = consts.tile([128, 2], f32)
            nc.sync.dma_start(out=bcol[:, 0:1], in_=Bd[0:128][:, None])
            nc.sync.dma_start(out=bcol[:, 1:2], in_=Bd[128:256][:, None])
            identity_bf = consts.tile([128, 128], bf16)
            nc.vector.tensor_copy(identity_bf, identity)
            wbc = consts.tile([128, C], f32)

            for _rep in range(repeat):
                # prime the ones column of every st slot (written once; the
                # per-group copies below never touch column NCH of a block)
                for _ in range(ST_BUFS):
                    stpr = stp.tile([128, GRP, NCH + 1], bf16, tag="st",
                                    name="stpr")
                    nc.vector.memset(stpr[:, :, NCH:NCH + 1], 1.0)

                x_tiles = {}
                wmb = {}
                offs_col = {}
                state = {"ce": 0}

                def load_half(h, bs_list):
                    hs = slice(h * 128, (h + 1) * 128)
                    for b in bs_list:
                        xt = res.tile([128, HW], bf16, tag=f"rxt{b}_{h}",
                                      name="rxt")
                        x_tiles[(b, h)] = xt
                        nc.gpsimd.dma_start(out=xt, in_=Xd[b, hs, :])

                def cov_half(h, bs_list, cov, alternate):
                    for b in bs_list:
                        xt = x_tiles[(b, h)]
                        for blk in groups:
                            pt = trp.tile([128, GRP, NCH], bf16, tag="pt",
                                          name="pt")
                            st = stp.tile([128, GRP, NCH + 1], bf16, tag="st",
                                          name="st")
                            for j, cidx in enumerate(blk):
                                kw = widths[cidx]
                                nc.tensor.transpose(
                                    pt[0:kw, j, :],
                                    xt[:, offs[cidx]:offs[cidx] + kw],
                                    identity_bf,
                                )
                            nblk = len(blk)
                            if alternate and state["ce"] % 2 == 1:
                                nc.scalar.copy(st[:, 0:nblk, 0:NCH],
                                               pt[:, 0:nblk, :])
                            else:
                                nc.vector.tensor_copy(st[:, 0:nblk, 0:NCH],
                                                      pt[:, 0:nblk, :])
                            state["ce"] += 1
                            for j, cidx in enumerate(blk):
                                kw = widths[cidx]
                                first = (b == 0) and (cidx == 0)
                                last = (b == BS - 1) and (cidx == NCHUNK - 1)
                                nc.tensor.matmul(
                                    cov,
                                    st[0:kw, j, 0:NCH],
                                    st[0:kw, j, 0:NCH + 1],
                                    start=first, stop=last,
                                )

                def start_allreduce(h, cov):
                    # Queue choice matters: the sync (SP) HWDGE ring carries
                    # only output stores, so a bounce DMA there would block
                    # them behind the collective wait. h=0 bounces ride the
                    # ACT HWDGE ring (idle at that point); h=1 bounces ride
                    # SWDGE, whose load traffic has drained by then.
                    dma = nc.scalar.dma_start if h == 0 else \
                        nc.gpsimd.dma_start
                    cc_in = statsp.tile([128, NCH + 1], f32, tag=f"cc{h}",
                                        name=f"cc{h}")
                    nc.vector.tensor_copy(cc_in, cov)
                    bounce_in = dram.tile([128, NCH + 1], f32, tag=f"bin{h}",
                                          name=f"bin{h}")
                    bounce_out = dram.tile([128, NCH + 1], f32, tag=f"bout{h}",
                                           name=f"bout{h}")
                    dma(out=bounce_in, in_=cc_in)
                    if single_core_sim:
                        dma(out=bounce_out, in_=bounce_in)
                    else:
                        nc.gpsimd.collective_compute(
                            "AllReduce",
                            mybir.AluOpType.add,
                            replica_groups=[list(range(NCORES))],
                            ins=[bounce_in.opt()],
                            outs=[bounce_out.opt()],
                        )
                    stats = statsp.tile([128, NCH + 1], f32, tag=f"stats{h}",
                                        name=f"stats{h}")
                    dma(out=stats, in_=bounce_out)
                    return stats

                def stats_ns(h, stats):
                    """All-reduced [cov | sums] -> wmb[h] (bf16 block-diag
                    whitening weights incl. weight scale) + offs_col[h].
                    PE: small matmuls; ACT: PSUM->SBUF copies; DVE: tiny
                    elementwise (emitted only after all streaming work, so
                    FIFO stalls here cost nothing)."""
                    # Two parallel dependency chains joining at sigN:
                    #   Sig path:   mean -> outer -> Sig = S/m - outer + epsI
                    #   trace path: diag(S) -> per-group trace -> rtr/srtr
                    # (trace(Sigma) = diag(S)/m + eps - mean^2, group-summed)
                    mean_col = statsp.tile([128, 1], f32, tag=f"mc{h}",
                                           name=f"mc{h}")
                    nc.vector.tensor_scalar(
                        out=mean_col, in0=stats[:, NCH:NCH + 1],
                        scalar1=inv_m, scalar2=None, op0=MULT)
                    dS = nss.tile([128, 128], f32, tag=f"dS{h}",
                                  name=f"dS{h}")
                    nc.vector.tensor_tensor(out=dS, in0=stats[:, 0:NCH],
                                            in1=identity, op=MULT)
                    pmr = nsp.tile([128, 128], f32, tag="nsp", name="pmr")
                    nc.tensor.transpose(pmr[0:1, 0:128], mean_col, identity)
                    mrow = statsp.tile([1, 128], f32, tag=f"mr{h}",
                                       name=f"mr{h}")
                    nc.scalar.copy(mrow, pmr[0:1, 0:128])
                    pouter = nsp.tile([128, 128], f32, tag="nsp",
                                      name="pouter")
                    nc.tensor.matmul(pouter, mrow, mrow, start=True, stop=True)
                    Sig = nss.tile([128, 128], f32, tag=f"sig{h}",
                                   name=f"sig{h}")
                    nc.vector.tensor_scalar(
                        out=Sig, in0=stats[:, 0:NCH],
                        scalar1=inv_m, scalar2=None, op0=MULT)
                    nc.vector.tensor_tensor(out=Sig, in0=Sig, in1=pouter,
                                            op=SUB)
                    nc.vector.tensor_tensor(out=Sig, in0=Sig, in1=epsI,
                                            op=ADD)
                    pdc = nsp.tile([128, 128], f32, tag="nsp", name="pdc")
                    nc.tensor.matmul(pdc[:, 0:1], dS, ones_col,
                                     start=True, stop=True)
                    msq = statsp.tile([128, 1], f32, tag=f"msq{h}",
                                      name=f"msq{h}")
                    nc.vector.tensor_tensor(out=msq, in0=mean_col,
                                            in1=mean_col, op=MULT)
                    dcol = statsp.tile([128, 1], f32, tag=f"dc{h}",
                                       name=f"dc{h}")
                    nc.vector.tensor_scalar(
                        out=dcol, in0=pdc[:, 0:1], scalar1=inv_m,
                        scalar2=EPS, op0=MULT, op1=ADD)
                    nc.vector.tensor_tensor(out=dcol, in0=dcol, in1=msq,
                                            op=SUB)
                    ptr = nsp.tile([128, 128], f32, tag="nsp", name="ptr")
                    nc.tensor.matmul(ptr[0:2, 0:1], gmask, dcol,
                                     start=True, stop=True)
                    traces = statsp.tile([2, 1], f32, tag=f"tr{h}",
                                         name=f"tr{h}")
                    nc.scalar.copy(traces, ptr[0:2, 0:1])
                    rsr = statsp.tile([2, 2], f32, tag=f"rsr{h}",
                                      name=f"rsr{h}")
                    nc.vector.reciprocal(rsr[:, 0:1], traces)
                    nc.scalar.sqrt(rsr[:, 1:2], rsr[:, 0:1])
                    prc = nsp.tile([128, 128], f32, tag="nsp", name="prc")
                    nc.tensor.matmul(prc[:, 0:2], gmaskT, rsr,
                                     start=True, stop=True)
                    rcols = statsp.tile([128, 2], f32, tag=f"rc{h}",
                                        name=f"rc{h}")
                    nc.scalar.copy(rcols, prc[:, 0:2])

                    # sigN' = -0.5 * Sigma/trace (the -0.5 of Newton-Schulz
                    # is folded in here so A|B can share one PSUM bank copy)
                    sigN = nss.tile([128, 128], f32, tag=f"sn{h}",
                                    name=f"sn{h}")
                    nc.vector.tensor_scalar(
                        out=sigN, in0=Sig, scalar1=rcols[:, 0:1],
                        scalar2=-0.5, op0=MULT, op1=MULT)

                    # Newton-Schulz: P <- 1.5 P + (P^2)(P sigN').
                    # A=P@P and B=P@sigN' land in one PSUM bank -> one DVE
                    # copy; the final matmul pair accumulates A@B + (1.5I)@P.
                    P = nss.tile([128, 128], f32, tag=f"P{h}", bufs=2,
                                 name=f"P{h}")
                    nc.vector.tensor_copy(P, identity)
                    for _t in range(T_ITERS):
                        psAB = nsp.tile([128, 256], f32, tag="nsp",
                                        name="psAB")
                        for k, sl in enumerate(blksl):
                            nc.tensor.matmul(
                                psAB[sl, sl], P[sl, sl], P[sl, sl],
                                start=True, stop=True,
                                tile_position=(64 * k, 64 * k))
                        for k, sl in enumerate(blksl):
                            nc.tensor.matmul(
                                psAB[sl, 128 + sl.start:128 + sl.stop],
                                P[sl, sl], sigN[sl, sl],
                                start=True, stop=True,
                                tile_position=(64 * k, 64 * k))
                        ABsb = nss.tile([128, 256], f32, tag=f"AB{h}", bufs=2,
                                        name="ABsb")
                        nc.vector.tensor_copy(ABsb, psAB)
                        psC = nsp.tile([128, 128], f32, tag="nsp", name="psC")
                        for k, sl in enumerate(blksl):
                            # keep each block's accumulate pair adjacent so a
                            # start=True of the other block can't disturb the
                            # has_written bits mid-accumulation
                            nc.tensor.matmul(
                                psC[sl, sl], ABsb[sl, sl],
                                ABsb[sl, 128 + sl.start:128 + sl.stop],
                                start=True, stop=False,
                                tile_position=(64 * k, 64 * k))
                            nc.tensor.matmul(
                                psC[sl, sl], id15[sl, sl], P[sl, sl],
                                start=False, stop=True,
                                tile_position=(64 * k, 64 * k))
                        Pn = nss.tile([128, 128], f32, tag=f"P{h}", bufs=2,
                                      name=f"Pn{h}")
                        # full-width copy: off-diagonal junk is never read
                        # (matmuls and wmb only touch the diag blocks)
                        nc.vector.tensor_copy(Pn, psC)
                        P = Pn

                    wm = nss.tile([128, 128], f32, tag=f"wm{h}",
                                  name=f"wm{h}")
                    nc.vector.tensor_scalar(
                        out=wm, in0=P, scalar1=rcols[:, 1:2],
                        scalar2=None, op0=MULT)
                    nc.vector.tensor_tensor(
                        out=wm, in0=wm, in1=wbc[:, h * 128:(h + 1) * 128],
                        op=MULT)
                    wb = nss.tile([128, 128], bf16, tag=f"wmb{h}",
                                  name=f"wmb{h}")
                    nc.vector.memset(wb, 0.0)
                    for sl in blksl:
                        nc.vector.tensor_copy(wb[sl, sl], wm[sl, sl])
                    wmb[h] = wb
                    poff = nsp.tile([128, 128], f32, tag="nsp", name="poff")
                    for k, sl in enumerate(blksl):
                        nc.tensor.matmul(
                            poff[sl, 0:1], wm[sl, sl], mean_col[sl, :],
                            start=True, stop=True,
                            tile_position=(64 * k, 64 * k))
                    oc = statsp.tile([128, 1], f32, tag=f"of{h}",
                                     name=f"of{h}")
                    nc.vector.tensor_tensor(
                        out=oc, in0=bcol[:, h:h + 1], in1=poff[:, 0:1],
                        op=SUB)
                    offs_col[h] = oc

                def apply_half(h, bs_list, alternate):
                    hs = slice(h * 128, (h + 1) * 128)
                    for b in bs_list:
                        xt = x_tiles[(b, h)]
                        stage = stg.tile([128, HW], f32, tag="stage",
                                         name="stage")
                        for k in range(HW // APPLY_N):
                            nsl = slice(k * APPLY_N, (k + 1) * APPLY_N)
                            pap = app.tile([128, APPLY_N], f32, tag="pap",
                                           name="pap")
                            nc.tensor.matmul(pap, wmb[h], xt[:, nsl],
                                             start=True, stop=True)
                            if alternate and state["ce"] % 2 == 1:
                                nc.scalar.add(stage[:, nsl], pap,
                                              offs_col[h])
                            else:
                                nc.vector.tensor_scalar(
                                    out=stage[:, nsl], in0=pap,
                                    scalar1=offs_col[h], scalar2=None,
                                    op0=ADD)
                            state["ce"] += 1
                        nc.sync.dma_start(out=Od[b, hs, :], in_=stage)

                # ---- schedule (emission order ~ intended execution order) --
                # The ar0 trigger must sit early in the gpsimd FIFO: SWDGE
                # descriptor generation for the loads is ring-capacity paced,
                # so only two h=1 loads are emitted ahead of it (their
                # desc-gen finishes well before cov0 lands). ar0 then
                # overlaps the h=1 streaming; NS0 + the first h=0 applies
                # slot in before the h=1 tail so stores start early; ar1 +
                # NS1 overlap the remaining h=0 applies.
                load_half(0, list(range(BS)))
                load_half(1, [0, 1])
                cov0 = covp.tile([128, NCH + 1], f32, tag="cov", name="cov0")
                cov_half(0, list(range(BS)), cov0, alternate=True)
                # build the weight broadcast here: emitting it earlier puts
                # it at the head of the PE/ACT FIFOs where its const DMAs
                # (slow behind the saturated load stream) stall streaming
                if _rep == 0:
                    pwb0 = nsp.tile([128, 256], f32, tag="nsp", name="pwb0")
                    nc.tensor.matmul(pwb0, ones_row, wrow, start=True,
                                     stop=True)
                    nc.scalar.copy(wbc, pwb0)
                stats0 = start_allreduce(0, cov0)
                load_half(1, list(range(2, BS)))
                cov1 = covp.tile([128, NCH + 1], f32, tag="cov", name="cov1")
                # ACT carries the ar0 bounce DMAs + stats0 copies meanwhile,
                # so the h=1 streaming copies stay DVE-only
                cov_half(1, [0, 1, 2, 3, 4, 5], cov1, alternate=False)
                stats_ns(0, stats0)
                apply_half(0, [0, 1], alternate=True)
                cov_half(1, [6, 7], cov1, alternate=False)
                stats1 = start_allreduce(1, cov1)
                # NS1 runs immediately after the ar1 fetch: stores only start
                # ~35us after ar1's data is in hand, so the engines' blocked
                # window here overlaps apply copies that aren't needed yet,
                # and apply1 then dovetails into the store stream with no gap
                stats_ns(1, stats1)
                apply_half(0, [2, 3, 4, 5, 6, 7], alternate=True)
                apply_half(1, list(range(BS)), alternate=True)

                if repeat > 1 and _rep < repeat - 1:
                    tc.strict_bb_all_engine_barrier()
    nc.compile()
    return nc


def kernel(X, weight, bias):
    from concourse.bass_utils import run_bass_kernel_spmd

    if "nc" not in _CACHE:
        _CACHE["nc"] = _build_nc()
    nc = _CACHE["nc"]

    X = np.ascontiguousarray(np.asarray(X, dtype=np.float32)).reshape(B, C, HW)
    w = np.ascontiguousarray(np.asarray(weight, dtype=np.float32)).reshape(C)
    bb = np.ascontiguousarray(np.asarray(bias, dtype=np.float32)).reshape(C)
    in_maps = [
        {"X": np.ascontiguousarray(X[i * BS:(i + 1) * BS]),
         "weight": w, "bias": bb}
        for i in range(NCORES)
    ]
    res = run_bass_kernel_spmd(nc, in_maps, core_ids=list(range(NCORES)))
    _CACHE["last_result"] = res
    out = np.concatenate([r["out"] for r in res.results], axis=0)
    return out.reshape(B, C, H, W)

